# revision 23
# baseline (speedup 1.0000x reference)
"""BiLSTM-CRF loss kernel for Trainium2 (8 NeuronCores, SPMD data-parallel).

Full inputs -> full scalar output. Sharding: batch 32 -> 4 rows/core x 8 cores.

v4 pipeline per core:
  gather embeddings (indirect DMA) -> PE-transpose -> input projections Gx
  (fp8 weights; bias add split between ACT and DVE) -> both LSTM directions as
  two independent, staggered dependency chains -> linear projection + exp
  emissions (dumped to host for the gold-path numerator) -> segmented CRF
  forward scan in linear space -> per-batch logZ.

LSTM step (per direction): identity-matmul injects the precomputed Gx block
into PSUM, 16 fp8 Whh tile matmuls accumulate the recurrent term, ONE sigmoid
covers all four gates (g-gate rows pre-scaled by 2: tanh(x) = 2 sigmoid(2x)-1).
Cell state is tracked halved (c' = c/2) so the update needs only
  u   = (sig_g - 0.5) * sig_i        [scalar_tensor_tensor, DVE]
  t1  = sig_f * c'_prev              [gpsimd]
  c'  = t1 + u                       [DVE]
  tc  = tanh(2 c')                   [ACT, free scale]
  h   = sig_o * tc                   [gpsimd, bf16 out]
Engines stay balanced: ACT 2 ops, DVE 2, Pool 2, PE 17 per dir per step.

CRF: t=1..511 split into 8 segments; each segment's running 9x9 product
P_s = prod diag(EM_t) M^T (M = exp(trans - log K)) is scanned in lockstep over
the local step index, 2 independent groups of 2 sequences for latency hiding.
One matmul [9,144] + one broadcast tensor_tensor per group per local step.
logZ_b = log(end^T P_7..P_0 (start*EM_0)) + 511 log K.
"""

import numpy as np
import ml_dtypes

VOCAB, EMB, HID, K, B, T = 30000, 256, 512, 9, 32, 512
H = HID // 2          # 256 per-direction hidden
NCORES = 8
BC = B // NCORES      # 4 batch rows per core
LOG_K = float(np.log(K))
# m-chunk order in the gates psum tile: [i0 i1 f0 f1 o0 o1 g0 g1]
MORDER = [0, 1, 2, 3, 6, 7, 4, 5]

NSEG = 8              # CRF time segments
SEGL = 64             # segment length (last one is 63)
NGRP = 2              # CRF lockstep groups (2 seqs each)

F8 = ml_dtypes.float8_e4m3
BF16 = ml_dtypes.bfloat16

_CACHE = {}


def _build_module(t_steps=T):
    import concourse.bacc as bacc
    import concourse.tile as tile
    import concourse.mybir as mybir
    from concourse import bass
    from concourse.masks import make_identity

    dt = mybir.dt
    AF = mybir.ActivationFunctionType
    ALU = mybir.AluOpType
    NT = t_steps * BC  # flattened (t, b) columns per core

    nc = bacc.Bacc("TRN2", target_bir_lowering=False, debug=False,
                   num_devices=NCORES)

    d_emb = nc.dram_tensor("embq", [VOCAB, EMB], dt.bfloat16, kind="ExternalInput").ap()
    d_tidx = nc.dram_tensor("tidx", [128, NT // 128], dt.int32, kind="ExternalInput").ap()
    d_wih = nc.dram_tensor("wih", [128, 2, 2, 8, 128], dt.float8e4, kind="ExternalInput").ap()
    d_whh = nc.dram_tensor("whh", [128, 2, 2, 8, 128], dt.float8e4, kind="ExternalInput").ap()
    d_gbias = nc.dram_tensor("gbias", [128, 2, 8], dt.float32, kind="ExternalInput").ap()
    d_wlin = nc.dram_tensor("wlin", [128, 4, K], dt.float8e4, kind="ExternalInput").ap()
    d_blin = nc.dram_tensor("blin", [K, 1], dt.float32, kind="ExternalInput").ap()
    d_et = nc.dram_tensor("et", [K, K], dt.bfloat16, kind="ExternalInput").ap()
    d_estart = nc.dram_tensor("estart", [K, 1], dt.float32, kind="ExternalInput").ap()
    d_eend = nc.dram_tensor("eend", [K, 1], dt.bfloat16, kind="ExternalInput").ap()
    d_h0 = nc.dram_tensor("h0q", [128, 2, 2, BC], dt.bfloat16, kind="ExternalInput").ap()
    d_c0 = nc.dram_tensor("c0i", [128, 2, 2, BC], dt.float32, kind="ExternalInput").ap()
    d_em = nc.dram_tensor("em", [K, NT], dt.float32, kind="ExternalOutput").ap()
    d_res = nc.dram_tensor("res", [1, BC], dt.float32, kind="ExternalOutput").ap()

    with tile.TileContext(nc) as tc:
        from contextlib import ExitStack
        with ExitStack() as ctx:
            pconst = ctx.enter_context(tc.tile_pool(name="pconst", bufs=1))

            # ---- persistent SBUF tensors ----
            sb_wih = pconst.tile([128, 2, 2, 8, 128], dt.float8e4)
            sb_whh = pconst.tile([128, 2, 2, 8, 128], dt.float8e4)
            sb_gbias = pconst.tile([128, 2, 8], dt.float32)
            sb_wlin = pconst.tile([128, 4, K], dt.float8e4)
            sb_blin = pconst.tile([K, 1], dt.float32)
            sb_et = pconst.tile([K, K], dt.bfloat16)
            sb_estart = pconst.tile([K, 1], dt.float32)
            sb_eend = pconst.tile([K, 1], dt.bfloat16)
            sb_tidx = pconst.tile([128, NT // 128], dt.int32)
            sb_h0 = pconst.tile([128, 2, 2, BC], dt.bfloat16)
            sb_c = pconst.tile([128, 2, 2, BC], dt.float32)   # running c/2 state
            sb_ident = pconst.tile([128, 128], dt.bfloat16)   # for PE transpose
            sb_ident8 = pconst.tile([128, 128], dt.float8e4)  # for Gx injection
            sb_xT = pconst.tile([128, 2, NT], dt.bfloat16)
            sb_gx = pconst.tile([128, 2, 8, NT], dt.bfloat16)
            sb_hsT = pconst.tile([128, 2, 2, NT], dt.bfloat16)  # [p, dir, khalf, col]
            sb_em = pconst.tile([K, NT], dt.float32)
            # CRF segment states (group-major so per-group slices are contiguous)
            sb_x = pconst.tile([K, NGRP, NSEG, 2, K], dt.bfloat16)
            sb_w = pconst.tile([K, BC], dt.bfloat16)           # CRF combine vecs
            sb_a0 = pconst.tile([K, BC], dt.bfloat16)
            sb_res = pconst.tile([1, BC], dt.float32)

            # spread input DMAs over both HWDGE queues; tidx first (gather dep)
            nc.sync.dma_start(out=sb_tidx[:], in_=d_tidx)
            nc.scalar.dma_start(out=sb_wih[:], in_=d_wih)
            nc.sync.dma_start(out=sb_whh[:], in_=d_whh)
            nc.scalar.dma_start(out=sb_gbias[:], in_=d_gbias)
            nc.sync.dma_start(out=sb_h0[:], in_=d_h0)
            nc.scalar.dma_start(out=sb_c[:], in_=d_c0)
            nc.sync.dma_start(out=sb_wlin[:], in_=d_wlin)
            nc.scalar.dma_start(out=sb_blin[:], in_=d_blin)
            nc.sync.dma_start(out=sb_et[:], in_=d_et)
            nc.scalar.dma_start(out=sb_estart[:], in_=d_estart)
            nc.sync.dma_start(out=sb_eend[:], in_=d_eend)
            make_identity(nc, sb_ident[:])
            make_identity(nc, sb_ident8[:])

            NCH = min(512, NT)

            # ---- phase A: gather + DMA-xbar transpose (PE stays free) ----
            # gather order interleaves both sequence ends so each LSTM
            # direction's first gx chunks are ready early
            nblk = NT // 128
            gorder = []
            for i in range(nblk // 2):
                gorder += [i, nblk - 1 - i]
            with tc.tile_pool(name="pgather", bufs=6) as pg, \
                 tc.tile_pool(name="pg_ps", bufs=4, space="PSUM") as pgp:
                for gi, i in enumerate(gorder):
                    xg = pg.tile([128, EMB], dt.bfloat16, tag="xg")
                    nc.gpsimd.indirect_dma_start(
                        out=xg[:],
                        out_offset=None,
                        in_=d_emb,
                        in_offset=bass.IndirectOffsetOnAxis(
                            ap=sb_tidx[:, i:i + 1], axis=0),
                    )
                    for k in range(2):
                        pst = pgp.tile([128, 128], dt.bfloat16, tag="pst")
                        nc.tensor.transpose(
                            out=pst[:], in_=xg[:, 128 * k:128 * (k + 1)],
                            identity=sb_ident[:])
                        if (gi + k) % 2 == 0:
                            nc.vector.tensor_copy(
                                sb_xT[:, k, 128 * i:128 * (i + 1)], pst[:])
                        else:
                            nc.scalar.activation(
                                sb_xT[:, k, 128 * i:128 * (i + 1)], pst[:],
                                AF.Copy)

            # ---- phase B: input projections for both directions ----
            # chunk order lets each LSTM direction start as soon as its first
            # gx chunk lands (fwd consumes low t first, rev high t first)
            nchunk = NT // NCH
            order = []
            for ci in range(nchunk):
                order.append((0, ci * NCH))
                order.append((1, (nchunk - 1 - ci) * NCH))
            with tc.tile_pool(name="pproj", bufs=4, space="PSUM") as ppp:
                for d, n0 in order:
                    for m in range(8):
                        if True:
                            psp = ppp.tile([128, NCH], dt.float32, tag="psp")
                            for k in range(2):
                                nc.tensor.matmul(
                                    psp[:], lhsT=sb_wih[:, d, k, m, :],
                                    rhs=sb_xT[:, k, n0:n0 + NCH],
                                    start=(k == 0), stop=(k == 1))
                            # bias add + bf16 cast; alternate ACT/DVE
                            if (m + n0 // NCH) % 2 == 0:
                                nc.scalar.activation(
                                    sb_gx[:, d, m, n0:n0 + NCH], psp[:],
                                    AF.Identity, bias=sb_gbias[:, d, m:m + 1])
                            else:
                                nc.vector.tensor_scalar_add(
                                    sb_gx[:, d, m, n0:n0 + NCH], psp[:],
                                    sb_gbias[:, d, m:m + 1])

            # ---- phase C: both LSTM recurrences, staggered chains ----
            # deep SBUF pool: keeps tile-reuse sem waits always-satisfied so
            # the ACT/DVE queues don't stall on 4-step-old consumers
            with tc.tile_pool(name="plstm", bufs=12) as pl, \
                 tc.tile_pool(name="plstm_ps", bufs=4, space="PSUM") as plp:
                for s in range(t_steps):
                    ps_d, sig_d, u_d, t1_d, tc_d = {}, {}, {}, {}, {}
                    for d in range(2):
                        t = s if d == 0 else t_steps - 1 - s
                        if s == 0:
                            rhs_prev = sb_h0[:, d]
                        else:
                            tp = t - 1 if d == 0 else t + 1
                            rhs_prev = sb_hsT[:, d, :, BC * tp:BC * (tp + 1)]
                        ps = plp.tile([128, 8, BC], dt.float32, tag=f"psl{d}")
                        nc.tensor.matmul(
                            ps[:], lhsT=sb_ident8[:],
                            rhs=sb_gx[:, d, :, BC * t:BC * (t + 1)],
                            start=True, stop=False)
                        for m in range(8):
                            for k in range(2):
                                nc.tensor.matmul(
                                    ps[:, m, :],
                                    lhsT=sb_whh[:, d, k, m, :],
                                    rhs=rhs_prev[:, k, :],
                                    start=False,
                                    stop=(m == 7 and k == 1))
                        sig = pl.tile([128, 8, BC], dt.float32, tag=f"sig{d}")
                        nc.scalar.activation(sig[:], ps[:], AF.Sigmoid)
                        ps_d[d], sig_d[d] = ps, sig
                    for d in range(2):
                        sig = sig_d[d]
                        u = pl.tile([128, 2, BC], dt.float32, tag=f"u{d}")
                        nc.vector.scalar_tensor_tensor(
                            out=u[:], in0=sig[:, 6:8, :], scalar=-0.5,
                            in1=sig[:, 0:2, :],
                            op0=ALU.add, op1=ALU.mult)
                        t1 = pl.tile([128, 2, BC], dt.float32, tag=f"t1{d}")
                        nc.gpsimd.tensor_mul(t1[:], sig[:, 2:4, :], sb_c[:, d])
                        u_d[d], t1_d[d] = u, t1
                    for d in range(2):
                        nc.vector.tensor_add(sb_c[:, d], t1_d[d][:], u_d[d][:])
                    for d in range(2):
                        # sigma(4 c') = sigma(2c); tanh(c) = 2 sigma(2c) - 1
                        tch = pl.tile([128, 2, BC], dt.float32, tag=f"tc{d}")
                        nc.scalar.activation(tch[:], sb_c[:, d], AF.Sigmoid,
                                             scale=4.0)
                        tc_d[d] = tch
                    for d in range(2):
                        # h/2 = (sigma(2c) - 0.5) * sigma(o); weights eat the 2x
                        t = s if d == 0 else t_steps - 1 - s
                        nc.vector.scalar_tensor_tensor(
                            out=sb_hsT[:, d, :, BC * t:BC * (t + 1)],
                            in0=tc_d[d][:], scalar=-0.5,
                            in1=sig_d[d][:, 4:6, :],
                            op0=ALU.add, op1=ALU.mult)

            # ---- phase D: feats -> EM (emissions; also dumped for host) ----
            with tc.tile_pool(name="pfeat_ps", bufs=4, space="PSUM") as pfp:
                for n0 in range(0, NT, NCH):
                    psf = pfp.tile([K, NCH], dt.float32, tag="psf")
                    for kk in range(4):
                        nc.tensor.matmul(
                            psf[:], lhsT=sb_wlin[:, kk, :],
                            rhs=sb_hsT[:, kk // 2, kk % 2, n0:n0 + NCH],
                            start=(kk == 0), stop=(kk == 3))
                    nc.scalar.activation(
                        sb_em[:, n0:n0 + NCH], psf[:], AF.Exp,
                        bias=sb_blin[:, 0:1])
            nc.sync.dma_start(out=d_em, in_=sb_em[:])

            # ---- phase E: segmented CRF scan ----
            em3 = sb_em[:].rearrange("j (t b) -> j t b", b=BC)
            with tc.tile_pool(name="pcrf", bufs=4) as pr, \
                 tc.tile_pool(name="pcrf_ps", bufs=3, space="PSUM") as prp:
                # init: X[s, g, b] = diag(EM[t=64s+1]) @ M^T  == M^T scaled
                # per-column?  No: A_t = diag(EM_t) M^T -> row j of A_t is
                # EM_t[j] * (M^T)[j,:] -> per-PARTITION scale: X = ET_bcast
                # with each partition j scaled by EM_t[j].
                for g in range(NGRP):
                    et_b = sb_et[:].unsqueeze(1).unsqueeze(1) \
                        .broadcast_to([K, NSEG, 2, K])
                    emi = em3[:, 1::SEGL, 2 * g:2 * g + 2]  # [K, 8, 2]
                    emi = emi.unsqueeze(3).broadcast_to([K, NSEG, 2, K])
                    nc.vector.tensor_mul(sb_x[:, g], et_b, emi)
                # lockstep scan l = 1..63
                for l in range(1, SEGL):
                    for g in range(NGRP):
                        ns = NSEG if l < SEGL - 1 else NSEG - 1
                        psx = prp.tile([K, NSEG, 2, K], dt.float32,
                                       tag=f"px{g}")
                        nc.tensor.matmul(psx[:, 0:ns], lhsT=sb_et[:],
                                         rhs=sb_x[:, g, 0:ns],
                                         start=True, stop=True)
                        emv = em3[:, l:l + 1 + (ns - 1) * SEGL:SEGL,
                                  2 * g:2 * g + 2]
                        emv = emv.unsqueeze(3).broadcast_to([K, ns, 2, K])
                        nc.vector.tensor_mul(sb_x[:, g, 0:ns], psx[:, 0:ns],
                                             emv)
            with tc.tile_pool(name="pcmb", bufs=4) as pr, \
                 tc.tile_pool(name="pcmb_ps", bufs=2, space="PSUM") as prp:
                # combine: w_b = P_0^T P_1^T ... P_7^T end  (right to left);
                # si outer so the 4 sequence chains interleave on PE/DVE
                for si in range(NSEG - 1, -1, -1):
                    for b in range(BC):
                        g, bb = b // 2, b % 2
                        pw = prp.tile([K, 1], dt.float32, tag=f"pw{b % 2}")
                        rhs = sb_eend[:, 0:1] if si == NSEG - 1 \
                            else sb_w[:, b:b + 1]
                        nc.tensor.matmul(pw[:], lhsT=sb_x[:, g, si, bb, :],
                                         rhs=rhs, start=True, stop=True)
                        nc.vector.tensor_copy(sb_w[:, b:b + 1], pw[:])
                # z_b = a0_b . w_b;  a0 = EM_0 * start
                nc.vector.tensor_scalar_mul(sb_a0[:], em3[:, 0, :],
                                            sb_estart[:, 0:1])
                for b in range(BC):
                    pz = prp.tile([1, 1], dt.float32, tag="pz")
                    nc.tensor.matmul(pz[:], lhsT=sb_a0[:, b:b + 1],
                                     rhs=sb_w[:, b:b + 1],
                                     start=True, stop=True)
                    nc.vector.tensor_copy(sb_res[0:1, b:b + 1], pz[:])
                lnz = pr.tile([1, BC], dt.float32, tag="lnz")
                nc.scalar.activation(lnz[:], sb_res[:], AF.Ln)
                nc.vector.tensor_scalar_add(
                    sb_res[:], lnz[:], float((t_steps - 1) * LOG_K))

            nc.sync.dma_start(out=d_res, in_=sb_res[:])

    nc.compile()
    return nc


def _prep_core_inputs(inputs, core, t_steps=T):
    """Host-side: slice batch shard + lay out tensors exactly as SBUF wants."""
    b0 = core * BC
    texts = np.asarray(inputs["texts"])[b0:b0 + BC, :t_steps]   # (BC, T)

    NT = t_steps * BC
    flat = texts.T.reshape(NT)                      # col c = t*BC + b
    tidx = flat.reshape(NT // 128, 128).T.astype(np.int32).copy()

    h0 = np.asarray(inputs["h0"])[:, b0:b0 + BC]    # (2, BC, 256)
    c0 = np.asarray(inputs["c0"])[:, b0:b0 + BC]
    # h is tracked halved on-device (weights carry the 2x)
    h0q = np.ascontiguousarray(
        h0.reshape(2, BC, 2, 128).transpose(3, 0, 2, 1) * 0.5).astype(BF16)
    # cell state is tracked halved on-device (tanh uses scale=2)
    c0i = np.ascontiguousarray(
        c0.reshape(2, BC, 2, 128).transpose(3, 0, 2, 1)).astype(np.float32) * 0.5

    return {"tidx": tidx, "h0q": h0q, "c0i": c0i}


def _prep_shared_inputs(inputs):
    embed = np.asarray(inputs["embed"])
    embq = embed.astype(BF16)

    def lhsT_pack(W, hscale=1.0):
        """W (1024, 256) -> [p, khalf, m, q]; g-gate rows are scaled by 2 so a
        single sigmoid computes every gate (tanh(x) = 2 sigmoid(2x) - 1).
        hscale=2 compensates the on-device h/2 hidden-state convention."""
        out = np.zeros((128, 2, 8, 128), np.float32)
        for k in range(2):
            for mi, mo in enumerate(MORDER):
                blk = W[128 * mo:128 * (mo + 1), 128 * k:128 * (k + 1)] * hscale
                if mi >= 6:
                    blk = blk * 2.0
                out[:, k, mi, :] = blk.T
        return out

    wih = np.stack([lhsT_pack(np.asarray(inputs["Wih_f"])),
                    lhsT_pack(np.asarray(inputs["Wih_r"]))], axis=1)
    whh = np.stack([lhsT_pack(np.asarray(inputs["Whh_f"]), 2.0),
                    lhsT_pack(np.asarray(inputs["Whh_r"]), 2.0)], axis=1)
    wih = np.ascontiguousarray(wih).astype(F8)
    whh = np.ascontiguousarray(whh).astype(F8)

    def bias_pack(bvec):
        out = np.stack([bvec[128 * mo:128 * (mo + 1)] for mo in MORDER])
        out = out.astype(np.float64)
        out[6:8] *= 2.0
        return out

    gbias = np.stack([bias_pack(np.asarray(inputs["b_f"])),
                      bias_pack(np.asarray(inputs["b_r"]))])
    gbias = np.ascontiguousarray(gbias.transpose(2, 0, 1)).astype(np.float32)

    W_lin = np.asarray(inputs["W_lin"])
    wlin = np.zeros((128, 4, K), np.float32)
    for kk in range(4):
        # x2 compensates the on-device h/2 hidden-state convention
        wlin[:, kk, :] = W_lin[:, 128 * kk:128 * (kk + 1)].T * 2.0
    wlin = wlin.astype(F8)

    blin = np.asarray(inputs["b_lin"]).reshape(K, 1).astype(np.float32)
    trans = np.asarray(inputs["trans"]).astype(np.float64)
    et = np.exp(trans - LOG_K).astype(BF16)
    estart = np.exp(np.asarray(inputs["start_trans"], np.float64)).reshape(K, 1).astype(np.float32)
    eend = np.exp(np.asarray(inputs["end_trans"], np.float64)).reshape(K, 1).astype(BF16)

    return {"embq": embq, "wih": wih, "whh": whh, "gbias": gbias,
            "wlin": wlin, "blin": blin, "et": et, "estart": estart,
            "eend": eend}


def host_combine(inputs, res_list, em_list, t_steps=T):
    """res_list[c] = (1, BC) logZ; em_list[c] = (K, NT) emissions exp(feats)."""
    tags = np.asarray(inputs["tags"])[:, :t_steps]
    start = np.asarray(inputs["start_trans"], np.float64)
    end = np.asarray(inputs["end_trans"], np.float64)
    trans = np.asarray(inputs["trans"], np.float64)

    logZ = np.concatenate([np.asarray(r, np.float64)[0] for r in res_list])

    em_sums = np.zeros(B, np.float64)
    tcol = np.arange(t_steps)
    for c in range(NCORES):
        lf = np.log(np.asarray(em_list[c], np.float64))  # (K, T*BC)
        for b in range(BC):
            tg = tags[c * BC + b]
            em_sums[c * BC + b] = lf[tg, tcol * BC + b].sum()

    tg = tags.T
    hostscore = start[tg[0]] + trans[tg[:-1], tg[1:]].sum(0) + end[tg[-1]]
    loss = -np.mean(em_sums + hostscore - logZ)
    return np.float32(loss)


def kernel(**inputs):
    from concourse.bass_utils import run_bass_kernel_spmd

    if "nc" not in _CACHE:
        _CACHE["nc"] = _build_module(T)
    nc = _CACHE["nc"]

    shared = _prep_shared_inputs(inputs)
    in_maps = []
    for c in range(NCORES):
        m = dict(shared)
        m.update(_prep_core_inputs(inputs, c))
        in_maps.append(m)

    out = run_bass_kernel_spmd(nc, in_maps, core_ids=list(range(NCORES)))
    res_list = [out.results[c]["res"] for c in range(NCORES)]
    em_list = [out.results[c]["em"] for c in range(NCORES)]
    return host_combine(inputs, res_list, em_list)
